# revision 8
# baseline (speedup 1.0000x reference)
import math

import jax
import jax.numpy as jnp
import numpy as np

C, AC, H, MID, NTR, ITERS = 128, 32, 4, 128, 4, 2
N, B = 256, 1

BF16 = jnp.bfloat16
F32 = jnp.float32


USE_BF16 = False  # bf16 operands: ~1e-3 rel err, no measured speed win on this backend


def _dot(a, w):
    if USE_BF16:
        return jnp.matmul(a.astype(BF16), w.astype(BF16), preferred_element_type=F32)
    return jnp.matmul(a, w)


def _ein(eq, a, b):
    if USE_BF16:
        return jnp.einsum(eq, a.astype(BF16), b.astype(BF16), preferred_element_type=F32)
    return jnp.einsum(eq, a, b)


def _ln(x, g, b):
    mu = jnp.mean(x, axis=-1, keepdims=True)
    var = jnp.mean(jnp.square(x - mu), axis=-1, keepdims=True)
    return (x - mu) * jax.lax.rsqrt(var + 1e-5) * g + b


def _tri_attn(x, mask, p, starting, skip_mask):
    x = _ln(x, p['ln_g'], p['ln_b'])
    sh = x.shape[:-1] + (AC, H)
    scale = 1.0 / math.sqrt(AC)
    q = _dot(x, p['wq'] * scale).reshape(sh)
    k = _dot(x, p['wk']).reshape(sh)
    v = _dot(x, p['wv']).reshape(sh)
    bias = _dot(x, p['wb'])
    g = jax.nn.sigmoid(_dot(x, p['wg']) + p['bg']).reshape(sh)
    # softmax axis kept innermost (k last) to avoid large transposes
    if starting:
        w = _ein('bijch,bikch->bijhk', q, k) + jnp.transpose(bias, (0, 2, 3, 1))[:, None]
        if not skip_mask:
            w = (w + 100.0) * mask[:, :, None, None, :] - 100.0
        w = jax.nn.softmax(w, axis=-1)
        o = _ein('bijhk,bikch->bijch', w, v) * g
    else:
        w = _ein('bijch,bkjch->bijhk', q, k) + jnp.transpose(bias, (0, 1, 3, 2))[:, :, None]
        if not skip_mask:
            w = (w + 100.0) * jnp.transpose(mask, (0, 2, 1))[:, None, :, None, :] - 100.0
        w = jax.nn.softmax(w, axis=-1)
        o = _ein('bijhk,bkjch->bijch', w, v) * g
    o = _dot(o.reshape(o.shape[:-2] + (AC * H,)), p['wo']) + p['bo']
    if not skip_mask:
        o = o * mask[..., None]
    return o


def _tri_mul(x, mask, p, eq, skip_mask):
    x = _ln(x, p['ln_g'], p['ln_b'])
    a = (_dot(x, p['wi']) + p['bi']) * jax.nn.sigmoid(_dot(x, p['wis']) + p['bis'])
    bb = (_dot(x, p['wj']) + p['bj']) * jax.nn.sigmoid(_dot(x, p['wjs']) + p['bjs'])
    if not skip_mask:
        a = a * mask[..., None]
        bb = bb * mask[..., None]
    out = _ein(eq, a, bb)
    out = _ln(out, p['ln2_g'], p['ln2_b'])
    out = _dot(out, p['wp']) + p['bp']
    out = out * jax.nn.sigmoid(_dot(x, p['ws']) + p['bs'])
    if not skip_mask:
        out = out * mask[..., None]
    return out


def _transition(x, p):
    x = _ln(x, p['ln_g'], p['ln_b'])
    x = jax.nn.relu(_dot(x, p['w1']) + p['b1'])
    return _dot(x, p['w2']) + p['b2']


def _stack(x2d, mask, params, skip_mask=False):
    for lp in params['layers']:
        x2d = x2d + _tri_attn(x2d, mask, lp['tas'], True, skip_mask)
        x2d = x2d + _tri_attn(x2d, mask, lp['tae'], False, skip_mask)
        x2d = x2d + _tri_mul(x2d, mask, lp['tmo'], 'bikc,bjkc->bijc', skip_mask)
        x2d = x2d + _tri_mul(x2d, mask, lp['tmi'], 'bkic,bkjc->bijc', skip_mask)
        x2d = x2d + _transition(x2d, lp['pt'])
        if not skip_mask:
            x2d = x2d * mask[..., None]
    x2d = _ln(x2d, params['ln_g'], params['ln_b'])
    if not skip_mask:
        x2d = x2d * mask[..., None]
    return x2d


_JIT_CACHE = {}


def _get_sharded_fn(skip_mask):
    """Build a jitted, 8-way i-sharded version of the stack."""
    key = ('fn', skip_mask)
    if key in _JIT_CACHE:
        return _JIT_CACHE[key]

    from jax.sharding import Mesh, NamedSharding, PartitionSpec as P

    devs = jax.devices()
    n = 8 if len(devs) >= 8 else len(devs)
    mesh = Mesh(np.array(devs[:n]).reshape(n), ('i',))

    xsh = NamedSharding(mesh, P(None, 'i', None, None))   # shard pair dim i
    msh = NamedSharding(mesh, P())                        # replicate mask
    psh = NamedSharding(mesh, P())                        # replicate params

    def fn(x2d, mask, params):
        return _stack(x2d, mask, params, skip_mask=skip_mask)

    jfn = jax.jit(
        fn,
        in_shardings=(xsh, msh, psh),
        out_shardings=NamedSharding(mesh, P(None, 'i', None, None)),
    )
    _JIT_CACHE[key] = (jfn, xsh, msh, psh)
    return _JIT_CACHE[key]


def kernel(x2d, mask, params):
    mask_np = np.asarray(mask, dtype=np.float32)
    skip_mask = bool(np.all(mask_np == 1.0))
    x2d = jnp.asarray(np.asarray(x2d), dtype=jnp.float32)
    mask = jnp.asarray(mask_np)
    params = jax.tree_util.tree_map(lambda a: jnp.asarray(np.asarray(a), jnp.float32), params)
    try:
        jfn, xsh, msh, psh = _get_sharded_fn(skip_mask)
        x_d = jax.device_put(x2d, xsh)
        m_d = jax.device_put(mask, msh)
        p_d = jax.device_put(params, psh)
        out = jfn(x_d, m_d, p_d)
        out = jax.block_until_ready(out)
        return np.asarray(out)
    except Exception:
        # fall back to single-device execution
        out = jax.jit(lambda x, m, p: _stack(x, m, p, skip_mask=False))(x2d, mask, params)
        return np.asarray(jax.block_until_ready(out))
